# revision 13
# baseline (speedup 1.0000x reference)
"""Trainium2 Bass kernel for ChannelLinearAttention.

Math (per batch element, V = queries.reshape(L, HE), all from the raw values):
    G      = V^T V                      [HE, HE]   (Gram over L)
    colsq  = diag(G);  r = 1/sqrt(colsq)
    vs     = sum_l V[l, :]              [HE]
    c      = (vs * r + eps) * r         [HE]
    W      = gamma * G * (r x r)        [HE, HE]
    part   = V @ W + gamma * vs         [L, HE]
    den    = HE + V @ c                 [L]
    out    = V + part / den[:, None]

Sharding: pure data parallel — B=16 batch elements, 2 per NeuronCore on 8 cores.
Matmuls run in bf16 (fp32 PSUM accumulation); the residual add of `queries`
stays in fp32, so global rel err ~1e-5.
"""

import numpy as np
from contextlib import ExitStack

import concourse.bass as bass
import concourse.tile as tile
from concourse import mybir
from concourse.bass_utils import run_bass_kernel_spmd
from concourse.masks import make_identity

FP32 = mybir.dt.float32
BF16 = mybir.dt.bfloat16
AF = mybir.ActivationFunctionType
ALU = mybir.AluOpType
AX = mybir.AxisListType


class _TC(tile.TileContext):
    """TileContext whose tail drain splits its semaphore waits.

    The walrus CoreV3 codegen on this toolchain rejects a CTRL/NOP-class
    instruction with more than 2 sync waits ("Too many sync wait commands").
    Tile's kernel-tail drain aggregates one wait per live semaphore, which
    exceeds that as soon as a kernel touches >2 queues. Split the waits over
    a chain of SP nops (same engine, in order, before the end barrier) so
    each instruction carries at most 2.
    """

    _MAX_WAITS = 1

    def _drain_and_barrier(self, tick_clock, wait_clock):
        from concourse.vector_clock import ScopedClock

        drain_inst = self.nc.sync.drain()
        wait_clock.add_sem_waits(
            drain_inst.ins, ScopedClock({None: tick_clock.global_clock})
        )
        si = drain_inst.ins.sync_info
        if si is not None and si.on_wait and len(si.on_wait) > self._MAX_WAITS:
            waits = list(si.on_wait)
            chunks = [waits[i:i + self._MAX_WAITS]
                      for i in range(0, len(waits), self._MAX_WAITS)]
            si.on_wait.clear()
            si.on_wait.extend(chunks[0])
            for ch in chunks[1:]:
                nop = self.nc.sync.nop(nofuse=True, hint="tail_drain_split")
                if nop.ins.sync_info is None:
                    nop.ins.sync_info = mybir.SyncInfo(on_wait=[], on_update=[])
                nop.ins.sync_info.on_wait.extend(ch)

        self.nc.all_engine_barrier()
        assert self.sems is not None
        popped = self.nc._tile_sem_poison_stack.pop()
        assert popped is self._sem_poison
        self.nc.clear_and_free_semaphores(list(self.sems.allocated().values()))
        self.nc.all_engine_barrier()

P = 128
B, L_FULL, H, E = 16, 4096, 8, 64
HE = H * E            # 512
N_CORES = 8
B_PER = B // N_CORES  # 2
EPS = 1e-6


def _split_sync_waits(nc, max_waits=1):
    """Walrus on this toolchain rejects instructions with more than one sync
    wait ("Too many sync wait commands"). Move extra waits onto preceding
    same-engine nops — the engine executes them in order, so semantics are
    preserved."""
    n = 0
    for f in nc.m.functions:
        for blk in f.blocks:
            new_insts = []
            for inst in blk.instructions:
                si = inst.sync_info
                waits = list(si.on_wait) if (si and si.on_wait) else []
                if len(waits) > max_waits:
                    extra, keep = waits[:-max_waits], waits[-max_waits:]
                    for i in range(0, len(extra), max_waits):
                        nop = mybir.InstNoOp(
                            name=f"I-waitsplit-{n}",
                            sync_info=mybir.SyncInfo(
                                on_wait=list(extra[i:i + max_waits]),
                                on_update=[]),
                            bass_nofuse=True,
                            engine=inst.engine,
                        )
                        n += 1
                        nc.register_instruction(nop, overwrite=True)
                        new_insts.append(nop)
                    si.on_wait.clear()
                    si.on_wait.extend(keep)
                new_insts.append(inst)
            blk.instructions[:] = new_insts


ALL_STAGES = frozenset({"ph1", "tail", "diag", "ph4a", "ph4b"})


def build_program(b_per=B_PER, L=L_FULL, num_devices=N_CORES, stages=ALL_STAGES,
                  repeat=1):
    NLT = L // P   # number of 128-row l-chunks
    NJ = HE // P   # 4 n-chunks

    nc = bass.Bass("TRN2", target_bir_lowering=False, debug=False,
                   num_devices=num_devices)
    q_d = nc.dram_tensor("q", [b_per, L, HE], FP32, kind="ExternalInput").ap()
    gam_d = nc.dram_tensor("gamma", [1, 1], FP32, kind="ExternalInput").ap()
    out_d = nc.dram_tensor("out", [b_per, L, HE], FP32, kind="ExternalOutput").ap()

    with _TC(nc) as tc, ExitStack() as ctx:
        _build(ctx, tc, out_d, q_d, gam_d, b_per, L, stages, repeat)
    _split_sync_waits(nc)
    return nc


def _build(ctx, tc, out_d, q_d, gam_d, b_per, L, stages=ALL_STAGES, repeat=1):
    nc = tc.nc
    NLT = L // P
    NJ = HE // P

    const = ctx.enter_context(tc.tile_pool(name="const", bufs=1))
    vpool = ctx.enter_context(tc.tile_pool(name="vpool", bufs=NLT))
    vbpool = ctx.enter_context(tc.tile_pool(name="vbpool", bufs=NLT))
    big = ctx.enter_context(tc.tile_pool(name="big", bufs=1))
    small = ctx.enter_context(tc.tile_pool(name="small", bufs=1))
    scr = ctx.enter_context(tc.tile_pool(name="scr", bufs=2))
    outp = ctx.enter_context(tc.tile_pool(name="outp", bufs=3))
    gps = ctx.enter_context(tc.tile_pool(name="gps", bufs=NJ, space="PSUM"))
    tps = ctx.enter_context(tc.tile_pool(name="tps", bufs=2, space="PSUM"))
    tailps = ctx.enter_context(tc.tile_pool(name="tailps", bufs=2, space="PSUM"))

    # ---------------- constants ----------------
    ident129 = const.tile([P, P + 1], BF16)    # [I_128 | ones] for transpose+colsum
    make_identity(nc, ident129[:, 0:P])
    nc.gpsimd.memset(ident129[:, P:P + 1], 1.0)
    i128b = const.tile([P, P], BF16)           # identity (diag masks)
    make_identity(nc, i128b)
    ones_kb = const.tile([P, P], BF16)         # all-ones, column-sum matmuls
    nc.gpsimd.memset(ones_kb, 1.0)
    ones_r1f = const.tile([1, P], FP32)
    nc.gpsimd.memset(ones_r1f, 1.0)

    gam_sb = const.tile([1, 1], FP32)
    nc.sync.dma_start(out=gam_sb, in_=gam_d[:, :])
    # broadcast gamma to all 128 partitions: [1,128]^T @ [1,1]
    gam_ps = tailps.tile([P, 1], FP32, tag="tail")
    nc.tensor.matmul(gam_ps, lhsT=ones_r1f, rhs=gam_sb, start=True, stop=True)
    gam_part = const.tile([P, 1], FP32)
    nc.scalar.copy(out=gam_part, in_=gam_ps)

    for b in [bb for _ in range(repeat) for bb in range(b_per)]:
        # ------------- phase 1: load, cast, Gram, transpose -------------
        vs_acc = small.tile([P, NJ], FP32, tag="vs_acc")
        nc.vector.memset(vs_acc, 0.0)
        vt_all = big.tile([P, NJ, L], BF16, tag="vt_all")    # V^T, vt[p,j,l] = V[l, 128j+p]
        w_all = big.tile([P, NJ, HE], BF16, tag="w_all")
        # G symmetry: block-row j only needs columns >= 128j
        g_tiles = [gps.tile([P, HE - j * P], FP32, tag="g", name=f"g_{b}_{j}")
                   for j in range(NJ)]

        v_tiles = []
        for k in range(NLT):
            v = vpool.tile([P, HE], FP32, tag="v", name=f"v_{b}_{k}")
            nc.sync.dma_start(out=v, in_=q_d[b, k * P:(k + 1) * P, :])
            v_tiles.append(v)
            if "ph1" not in stages:
                continue
            vb = vbpool.tile([P, HE], BF16, tag="vb", name=f"vb_{b}_{k}")
            nc.scalar.copy(out=vb, in_=v)

            # Gram, upper triangle: G[128j+m, n>=128j] += V[l,128j+m] V[l,n]
            for j in range(NJ):
                nc.tensor.matmul(g_tiles[j], lhsT=vb[:, j * P:(j + 1) * P],
                                 rhs=vb[:, j * P:], start=(k == 0),
                                 stop=(k == NLT - 1))
            # transpose blocks (plus ones-column => per-tile column sums)
            for pr in range(NJ // 2):
                t = tps.tile([P, 2, P + 1], FP32, tag="tp", name=f"t_{b}_{k}_{pr}")
                for jj in range(2):
                    j = 2 * pr + jj
                    nc.tensor.matmul(t[:, jj, :], lhsT=vb[:, j * P:(j + 1) * P],
                                     rhs=ident129, start=True, stop=True)
                nc.vector.tensor_copy(
                    out=vt_all[:, 2 * pr:2 * pr + 2, k * P:(k + 1) * P],
                    in_=t[:, :, 0:P])
                nc.vector.tensor_add(out=vs_acc[:, 2 * pr:2 * pr + 2],
                                     in0=vs_acc[:, 2 * pr:2 * pr + 2],
                                     in1=t[:, :, P])

        # ------------- phase 2/3: tail math -------------
        if "tail" not in stages:
            nc.sync.dma_start(out=out_d[b, 0:P, :], in_=v_tiles[0])
            continue
        # colsq[128j+p] = G[128j+p, 128j+p]: masked row-sum of G's diag block
        colsq4 = small.tile([P, NJ], FP32, tag="colsq4")
        dscr = scr.tile([P, NJ, P], FP32, tag="dscr", name=f"dscr_{b}")
        for j in range(NJ):
            nc.vector.scalar_tensor_tensor(out=dscr[:, j, :],
                                           in0=g_tiles[j][:, 0:P],
                                           scalar=1.0, in1=i128b,
                                           op0=ALU.mult, op1=ALU.mult,
                                           accum_out=colsq4[:, j:j + 1])
        norm4 = small.tile([P, NJ], FP32, tag="norm4")
        nc.scalar.sqrt(out=norm4, in_=colsq4)
        r4 = small.tile([P, NJ], FP32, tag="r4")
        nc.vector.reciprocal(out=r4, in_=norm4)
        # c = (vs*r + eps) * r
        c4 = small.tile([P, NJ], FP32, tag="c4")
        nc.vector.tensor_mul(out=c4, in0=vs_acc, in1=r4)
        nc.vector.tensor_scalar(out=c4, in0=c4, scalar1=EPS, scalar2=None,
                                op0=ALU.add)
        nc.vector.tensor_mul(out=c4, in0=c4, in1=r4)
        # sc4 = gamma * r  (per-partition scale for W rows)
        sc4 = small.tile([P, NJ], FP32, tag="sc4")
        nc.vector.tensor_scalar(out=sc4, in0=r4, scalar1=gam_part, scalar2=None,
                                op0=ALU.mult)

        # diagonal expansions: xdiag[p, j*128+f] = x[128j+p] * (f==p)
        if "diag" not in stages:
            nc.sync.dma_start(out=out_d[b, 0:P, :], in_=v_tiles[0])
            continue
        rdiag = small.tile([P, NJ, P], BF16, tag="rdiag")
        cdiag = small.tile([P, NJ, P], BF16, tag="cdiag")
        vsgdiag = small.tile([P, NJ, P], BF16, tag="vsgdiag")
        for j in range(NJ):
            nc.vector.tensor_scalar(out=rdiag[:, j, :], in0=i128b,
                                    scalar1=r4[:, j:j + 1], scalar2=None,
                                    op0=ALU.mult)
            nc.vector.tensor_scalar(out=cdiag[:, j, :], in0=i128b,
                                    scalar1=c4[:, j:j + 1], scalar2=None,
                                    op0=ALU.mult)
            nc.vector.tensor_scalar(out=vsgdiag[:, j, :], in0=i128b,
                                    scalar1=vs_acc[:, j:j + 1], scalar2=gam_part,
                                    op0=ALU.mult, op1=ALU.mult)

        # column-sum matmuls -> broadcast rows
        rbc_ps = tailps.tile([P, HE], FP32, tag="tail", name=f"rbc_{b}")
        nc.tensor.matmul(rbc_ps, lhsT=ones_kb, rhs=rdiag, start=True, stop=True)
        r_bcast = big.tile([P, HE], FP32, tag="r_bcast")   # r_bcast[p,n] = r[n]
        nc.vector.tensor_copy(out=r_bcast, in_=rbc_ps)

        cbc_ps = tailps.tile([P, HE], FP32, tag="tail", name=f"cbc_{b}")
        nc.tensor.matmul(cbc_ps, lhsT=ones_kb, rhs=cdiag, start=True, stop=True)
        c_bcast = big.tile([P, HE], FP32, tag="c_bcast")   # c_bcast[p,n] = c[n]
        nc.vector.tensor_copy(out=c_bcast, in_=cbc_ps)

        vsg_ps = tailps.tile([P, HE], FP32, tag="tail", name=f"vsg_{b}")
        nc.tensor.matmul(vsg_ps, lhsT=ones_kb, rhs=vsgdiag, start=True, stop=True)
        vsg_bcast = big.tile([P, HE], FP32, tag="vsg_bcast")  # [p,n] = gamma*vs[n]
        nc.vector.tensor_copy(out=vsg_bcast, in_=vsg_ps)

        # W[128j+p, n>=128j] = (gamma*r[128j+p]) * G[128j+p, n] * r[n]
        for j in range(NJ):
            nc.vector.scalar_tensor_tensor(out=w_all[:, j, j * P:],
                                           in0=g_tiles[j],
                                           scalar=sc4[:, j:j + 1],
                                           in1=r_bcast[:, j * P:],
                                           op0=ALU.mult, op1=ALU.mult)
        # lower-triangle blocks of W by transposing the upper ones (W = W^T)
        for j in range(1, NJ):
            for jp in range(j):
                wt_ps = tailps.tile([P, P], FP32, tag="tail",
                                    name=f"wt_{b}_{j}_{jp}")
                nc.tensor.matmul(wt_ps, lhsT=w_all[:, jp, j * P:(j + 1) * P],
                                 rhs=i128b, start=True, stop=True)
                nc.vector.tensor_copy(out=w_all[:, j, jp * P:(jp + 1) * P],
                                      in_=wt_ps)

        # ------------- phase 4a: den/tailor (independent of part matmuls) ----
        if "ph4a" not in stages:
            nc.sync.dma_start(out=out_d[b, 0:P, :], in_=v_tiles[0])
            continue
        den_all = small.tile([P, NLT], FP32, tag="den_all")
        tailor_all = small.tile([P, NLT], FP32, tag="tailor_all")
        for i in range(NLT):
            scr512 = scr.tile([P, HE], FP32, tag="scr512", name=f"ttr_{b}_{i}")
            # den_raw[i] = sum_n V[l, n] * c[n]   (walrus rejects
            # tensor_tensor_reduce here, so use stt with accum_out)
            nc.vector.scalar_tensor_tensor(out=scr512, in0=v_tiles[i],
                                           scalar=1.0, in1=c_bcast,
                                           op0=ALU.mult, op1=ALU.mult,
                                           accum_out=den_all[:, i:i + 1])
            # tailor = 1 / (HE + den_raw)
            nc.vector.tensor_scalar(out=den_all[:, i:i + 1],
                                    in0=den_all[:, i:i + 1],
                                    scalar1=float(HE), scalar2=None, op0=ALU.add)
            nc.vector.reciprocal(out=tailor_all[:, i:i + 1],
                                 in_=den_all[:, i:i + 1])

        # ------------- phase 4b: part matmuls + epilogue -------------
        if "ph4b" not in stages:
            nc.sync.dma_start(out=out_d[b, 0:P, :], in_=v_tiles[0])
            continue
        for i in range(NLT):
            pp = tps.tile([P, HE], FP32, tag="tp", name=f"pp_{b}_{i}")
            for j in range(NJ):
                nc.tensor.matmul(pp, lhsT=vt_all[:, j, i * P:(i + 1) * P],
                                 rhs=w_all[:, j, :], start=(j == 0),
                                 stop=(j == NJ - 1))
            # part += gamma*vs[n]  (broadcast add into PSUM on DVE)
            nc.vector.tensor_add(out=pp, in0=pp, in1=vsg_bcast)
            # out = V + part*tailor
            out_t = outp.tile([P, HE], FP32, tag="out_t", name=f"o_{b}_{i}")
            nc.vector.scalar_tensor_tensor(out=out_t, in0=pp,
                                           scalar=tailor_all[:, i:i + 1],
                                           in1=v_tiles[i],
                                           op0=ALU.mult, op1=ALU.add)
            nc.sync.dma_start(out=out_d[b, i * P:(i + 1) * P, :], in_=out_t)


_PROGRAM_CACHE = {}


def _get_program():
    key = (B_PER, L_FULL)
    if key not in _PROGRAM_CACHE:
        _PROGRAM_CACHE[key] = build_program()
    return _PROGRAM_CACHE[key]


def kernel(queries, keys=None, values=None, attn_mask=None, gamma=None, **kwargs):
    queries = np.ascontiguousarray(np.asarray(queries, dtype=np.float32))
    gamma_np = np.asarray(gamma, dtype=np.float32).reshape(1, 1)
    Bq, Lq, Hq, Eq = queries.shape
    assert (Bq, Lq, Hq, Eq) == (B, L_FULL, H, E)

    qr = queries.reshape(B, L_FULL, HE)
    in_maps = [
        {"q": np.ascontiguousarray(qr[i * B_PER:(i + 1) * B_PER]),
         "gamma": gamma_np}
        for i in range(N_CORES)
    ]
    nc = _get_program()
    res = run_bass_kernel_spmd(nc, in_maps, core_ids=list(range(N_CORES)))
    out = np.concatenate([np.asarray(res.results[i]["out"])
                          for i in range(N_CORES)], axis=0)
    return out.reshape(B, L_FULL, H, E).astype(np.float32)
